# revision 60
# baseline (speedup 1.0000x reference)
"""Causal self-attention (B=2, T=2048, C=1024, H=16) on 8 trn2 NeuronCores.

Sharding: core c = (batch b = c // 4, head-group g = c % 4). Each core
computes, for its batch, QKV for heads [4g, 4g+4), causal attention, and a
partial output projection through rows [256g, 256g+256) of W_proj. The host
sums the 4 partial projections per batch (tensor-parallel unshard) and adds
b_proj.

Per-core kernel structure (all matmul inputs bf16, fp32 PSUM):
  - qk^T is produced transposed ([channel, t]) so attention scores need no
    input transposes (contraction over d=64 sits on the partition axis).
  - Scores are computed TRANSPOSED (S^T[k, q] tiles): exp(S^T) is directly
    the P^T operand the PV matmul needs. The two heads of a pair go to the
    two halves of a [128, 2, 512] PSUM tile (one bank per head) via
    row-packed K=64 matmuls at tile_position rows 0/64 (concurrent).
  - Causal masking: for diagonal k-blocks the dead triangle of exp(S^T)
    is zeroed in SBUF by a DVE tril-multiply (no PE mask matmuls; PV then
    accumulates exact zeros for dead keys).
  - Diagonal k-block tiles are column-restricted to the causally-live
    query range (scores, exp and PV all skip the dead columns).
  - One ACT exp per (pair, k-block) covers both heads ([128, 2, live]);
    splitting per head costs more in ACT instruction overhead than it
    saves in PV latency (measured).
  - V carries an appended ones column (lhsT [128, 65]) so the PV matmul
    accumulates the softmax denominator as row 64 of y^T_aug for free.
  - Normalization: PSUM->SBUF copy of y_aug (bf16), then the denominator
    row is broadcast across partitions with a K=1 ones-matmul on the PE,
    then a DVE reciprocal from PSUM and per-head multiplies on gpsimd/DVE.
    (All broadcast alternatives measured worse on HW: DVE cannot read
    nonzero partition offsets, gpsimd partition_broadcast and DVE
    stream_shuffle are an order of magnitude slower than they look.)
    The chain is deferred into later attention steps as fillers.
  - y lands transposed ([d, q]), exactly the lhsT the projection needs.
    Projection partials are written to DRAM in bf16 (host sums in fp32).
  - PE is the bottleneck engine; QKV/V/projection matmul groups are
    emitted as fillers BETWEEN attention steps so the PE queue never
    drains. p1's first score stages are pre-staged at the tail of p0
    (the p-boundary is ACT-throughput-bound).

Scheduling against the hardware (all measured via NTFF traces):
  - HAM clock gate: the PE runs at 1.2GHz until it has been busy ~3.4us,
    then 2.4GHz. NWARM scratch matmuls at program start keep it busy (and
    end warm) while the prologue DMAs land, cutting first-matmul latency
    from ~17.7us to ~15.3us and removing the cold-clock penalty.
  - DMA rings: only sync (~197GB/s) and scalar (~117GB/s) hardware DGE
    rings exist; the gpsimd ring is software-DGE and poisons the whole
    chip (~20% slowdown) even for small transfers. A ring round-robins
    among ALL outstanding descriptors, so the prologue keeps the critical
    bytes (x chunk 0 on sync; j0/j2 then wv on scalar) in front and defers
    chunk 1 to a filler.
  - The final projection chunk pre-accumulates its p2=0 halves at
    background scheduler priority during the last norm chain (keeps HAM
    warm with real work), finishing with p2=1 after the last norm_mul.
    Tail drains split across ACT/DVE; all stores ride the sync ring whose
    engine queue has no drains to block.
"""

import sys
from collections import deque

for _p in ("/opt/trn_rl_repo",):
    if _p not in sys.path:
        sys.path.insert(0, _p)

import numpy as np
import ml_dtypes

import concourse.bass as bass
import concourse.tile as tile
from concourse import bacc, mybir
from concourse.bass_utils import run_bass_kernel_spmd

BF16 = mybir.dt.bfloat16
F32 = mybir.dt.float32
NP_BF16 = ml_dtypes.bfloat16

B, T, C = 2, 2048, 1024
H, D = 16, 64
N_CORES = 8
CT = C // 128   # 8 contraction tiles
TQ = T // 128   # 16 key blocks
QC = T // 512   # 4 query chunks
SCALE = 1.0 / np.sqrt(D)
NWARM = 18  # warm-up matmuls: keep PE busy (and HAM un-throttled) while the
            # prologue DMAs land, so real matmuls start at 2.4GHz

_compiled = None


def _build_nc(dbg=False):
    nc = bacc.Bacc("TRN2", target_bir_lowering=False, debug=False,
                   enable_asserts=False)
    if dbg:
        dbg_qkT = nc.dram_tensor("dbg_qkT", [128, 4, T], BF16, kind="ExternalOutput")
        dbg_v = nc.dram_tensor("dbg_v", [128, TQ, 4, 65], BF16, kind="ExternalOutput")
        dbg_yT = nc.dram_tensor("dbg_yT", [128, 2, T], BF16, kind="ExternalOutput")

    # pre-swizzled on host so every load is contiguous per partition
    xT_d = nc.dram_tensor("xT", [QC, 128, CT, 512], BF16, kind="ExternalInput")
    wqk_d = nc.dram_tensor("wqk", [128, 4, CT, 128], BF16, kind="ExternalInput")
    wv_d = nc.dram_tensor("wv", [128, CT, 256], BF16, kind="ExternalInput")
    wp_d = nc.dram_tensor("wp", [128, 2, C], BF16, kind="ExternalInput")
    bqk_d = nc.dram_tensor("bqk", [128, 4], F32, kind="ExternalInput")
    bv_d = nc.dram_tensor("bv", [128, 256], BF16, kind="ExternalInput")
    tril_d = nc.dram_tensor("tril", [128, 128], BF16, kind="ExternalInput")
    out_d = nc.dram_tensor("out", [T, C], BF16, kind="ExternalOutput")

    Exp = mybir.ActivationFunctionType.Exp

    with tile.TileContext(nc) as tc:
        with (
            tc.tile_pool(name="const", bufs=1) as cpool,
            tc.tile_pool(name="qkT", bufs=1) as qkpool,
            tc.tile_pool(name="vbuf", bufs=1) as vpool,
            tc.tile_pool(name="ybuf", bufs=1) as ypool,
            tc.tile_pool(name="pt", bufs=4) as ptpool,
            tc.tile_pool(name="norm", bufs=2) as npool,
            tc.tile_pool(name="ostage", bufs=4) as opool,
            tc.tile_pool(name="mmps", bufs=2, space="PSUM") as mmps,
            tc.tile_pool(name="sps", bufs=2, space="PSUM") as sps,
            tc.tile_pool(name="accps", bufs=2, space="PSUM") as accps,
        ):
            # ---- constants / weights ----
            # xT_s is chunk-major so chunk DMAs land contiguously; wqk_s is
            # j-major so each j-block is one contiguous DMA.
            xT_s = cpool.tile([128, QC, CT, 512], BF16)
            wqk_s = cpool.tile([128, 4, CT, 128], BF16)
            wv_s = cpool.tile([128, CT, 256], BF16)
            wp_s = cpool.tile([128, 2, C], BF16)
            bqk_s = cpool.tile([128, 4], F32)
            bv_s = cpool.tile([128, 256], BF16)
            tril_s = cpool.tile([128, 128], BF16)
            ones_s = cpool.tile([128, 64], BF16)
            scratch = cpool.tile([128, 512], BF16)

            # prologue loads on the two hardware DGE rings only (sync ~197GB/s,
            # scalar ~117GB/s; the gpsimd ring is software-DGE and slow).
            # sync: x chunk 0 (gates the first matmuls) then pair-1 weights;
            # scalar: small constants, then j0/j2 (first scores), then wv
            # (first PV consume). x chunk 1 is DMA'd from a filler later.
            # NOTE: each ring round-robins among its outstanding descriptors
            # (~197GB/s sync, ~117GB/s scalar aggregate).
            nc.sync.dma_start(out=xT_s[:, 0], in_=xT_d.ap()[0])
            nc.scalar.dma_start(out=bqk_s[:], in_=bqk_d.ap()[:])
            nc.scalar.dma_start(out=tril_s[:], in_=tril_d.ap()[:])
            for j in (0, 2):  # pair-0's Q/K blocks first
                nc.scalar.dma_start(out=wqk_s[:, j], in_=wqk_d.ap()[:, j])
            # pair-1 weights share the sync ring with x0 (the gpsimd ring is
            # software-DGE: even 0.5MB on it slows the whole chip down)
            nc.sync.dma_start(out=wqk_s[:, 1], in_=wqk_d.ap()[:, 1])
            nc.sync.dma_start(out=wqk_s[:, 3], in_=wqk_d.ap()[:, 3])
            nc.scalar.dma_start(out=wv_s[:], in_=wv_d.ap()[:])
            nc.scalar.dma_start(out=bv_s[:], in_=bv_d.ap()[:])
            nc.scalar.dma_start(out=wp_s[:], in_=wp_d.ap()[:])

            nc.vector.memset(ones_s[:], 1.0)
            nc.vector.memset(scratch[:], 0.125)

            warm = cpool.tile([128, 1], F32)
            nc.vector.memset(warm[:], 0.0)
            nc.scalar.activation(warm[:], warm[:], Exp)

            # warm-up matmuls: PE runs on scratch data while the prologue
            # DMAs land (HAM un-throttles after ~3.4us of sustained activity)
            warm_ps = mmps.tile([128, 512], F32, tag="mm")
            for _ in range(NWARM):
                nc.tensor.matmul(warm_ps[:], scratch[:, 0:128], scratch[:],
                                 start=True, stop=True)

            qkT_s = qkpool.tile([128, 4, T], BF16)
            v_s = vpool.tile([128, TQ, 4, 65], BF16)
            nc.vector.memset(v_s[:, :, :, 64:65], 1.0)
            yT_s = ypool.tile([128, 2, T], BF16)

            # ---- emission helpers (work groups used directly or as fillers) ----
            def dma_chunk(t4):
                nc.sync.dma_start(out=xT_s[:, t4], in_=xT_d.ap()[t4])

            Identity = mybir.ActivationFunctionType.Identity

            def qkv_half(j, t4, h):
                # jtile 0: Q heads {0,1}; 1: Q {2,3}; 2: K {0,1}; 3: K {2,3}
                # emitted as two filler halves (h=0 allocates, h=1 drains).
                # Chunk-0 drains on ACT (idle in the qc0 era, DVE crowded);
                # later chunks on DVE: a 0.7us ACT drain between two exps on
                # the strict-FIFO ACT queue delays the PV that waits on exp.
                ps = qkv_half.ps if h else mmps.tile([128, 512], F32, tag="mm")
                qkv_half.ps = ps
                for i in range(4 * h, 4 * h + 4):
                    nc.tensor.matmul(
                        ps[:],
                        wqk_s[:, j, i, :],
                        xT_s[:, t4, i, :],
                        start=(i == 0), stop=(i == CT - 1),
                    )
                if h:
                    dst = qkT_s[:, j, 512 * t4:512 * (t4 + 1)]
                    if t4 == 0:
                        # qc0 era: ACT is idle, DVE is the crowded engine
                        nc.scalar.activation(dst, ps[:], Identity,
                                             bias=bqk_s[:, j:j + 1])
                    else:
                        nc.vector.tensor_scalar_add(dst, ps[:], bqk_s[:, j:j + 1])

            def qkv_group(j, t4):
                qkv_half(j, t4, 0)
                qkv_half(j, t4, 1)

            def v_group(t):
                # bias lands in the drain (tensor_tensor add with bv_s), so
                # the PE does only the CT contraction matmuls.
                ps = mmps.tile([128, 256], F32, tag="mm")
                for i in range(CT):
                    nc.tensor.matmul(
                        ps[:],
                        xT_s[:, t // 4, i, 128 * (t % 4):128 * (t % 4 + 1)],
                        wv_s[:, i, :],
                        start=(i == 0), stop=(i == CT - 1),
                    )
                dst = v_s[:, t, :, 0:64]
                src = ps[:].rearrange("p (h d) -> p h d", h=4)
                bvv = bv_s[:].rearrange("p (h d) -> p h d", h=4)
                nc.vector.tensor_add(dst, src, bvv)

            def proj_half(t, n, o_t):
                ps = mmps.tile([128, 512], F32, tag="mm")
                for p2 in range(2):
                    nc.tensor.matmul(
                        ps[:],
                        yT_s[:, p2, 128 * t:128 * (t + 1)],
                        wp_s[:, p2, 512 * n:512 * (n + 1)],
                        start=(p2 == 0), stop=(p2 == 1),
                    )
                dst = o_t[:, 512 * n:512 * (n + 1)]
                if t >= 12:
                    nc.scalar.copy(dst, ps[:])
                else:
                    nc.vector.tensor_copy(dst, ps[:])
                # whole-block store on the sync ring only: ACT-ring issues
                # cost ~0.6us of ACT-queue time that delays exps (strict
                # FIFO), and fewer issues keep the sync queue light.
                if n == 1:
                    nc.sync.dma_start(out=out_d.ap()[128 * t:128 * (t + 1), :],
                                      in_=o_t[:])

            def proj_group(t):
                o_t = opool.tile([128, C], BF16, tag="o")
                proj_half(t, 0, o_t)
                proj_half(t, 1, o_t)

            def proj_fillers(t):
                """proj group as two filler-granular halves sharing one o_t."""
                box = {}

                def h(n, t=t, box=box):
                    if n == 0:
                        box["o"] = opool.tile([128, C], BF16, tag="o", name="o_t")
                    proj_half(t, n, box["o"])

                return [lambda: h(0), lambda: h(1)]

            NODL = (9, 9)
            fillers = deque()  # (deadline (qc, p), fn)

            def emit_filler(n=1):
                for _ in range(n):
                    if fillers:
                        fillers.popleft()[1]()

            def flush_due(key):
                """Emit every queued filler whose deadline is <= key."""
                keep = deque()
                while fillers:
                    dl, fn = fillers.popleft()
                    if dl <= key:
                        fn()
                    else:
                        keep.append((dl, fn))
                fillers.extend(keep)

            # ---- prologue: pair-0's Q/K j-tiles. v_group(0) is emitted
            # inside the (0,0) step after the score stages (wv lands late;
            # stages must not queue behind it on the in-order PE queue) ----
            qkv_group(0, 0)
            qkv_group(2, 0)

            # ---- attention: S^T tiles [k-block, q-chunk], flash over k ----
            for qc in range(QC):
                # stage work for later chunks (see scheduling notes in header)
                if qc == 0:
                    # order matters: v_group(t) must be emitted before
                    # consume(t) pops it (pops are 2 per consume), and the
                    # qkv halves slot into the wv-DMA wait window.
                    fillers.append(((0, 1), lambda: qkv_half(1, 0, 0)))
                    fillers.append(((0, 1), lambda: v_group(1)))
                    fillers.append(((0, 1), lambda: v_group(2)))
                    fillers.append(((0, 1), lambda: qkv_half(1, 0, 1)))
                    fillers.append(((0, 1), lambda: v_group(3)))
                    fillers.append(((0, 1), lambda: qkv_half(3, 0, 0)))
                    fillers.append(((0, 1), lambda: qkv_half(3, 0, 1)))
                if qc + 1 < QC:
                    dl = (qc + 1, 0)
                    fillers.append((dl, lambda t4=qc + 1: dma_chunk(t4)))
                    for j in range(4):
                        fillers.append((dl, lambda j=j, t4=qc + 1: qkv_half(j, t4, 0)))
                        fillers.append((dl, lambda j=j, t4=qc + 1: qkv_half(j, t4, 1)))
                    # V for the next chunk; the last chunk's V groups are
                    # deferred into qc3 itself (it is filler-starved).
                    if qc + 1 < QC - 1:
                        for t in range(4 * (qc + 1), 4 * (qc + 2)):
                            fillers.append((dl, lambda t=t: v_group(t)))
                if qc == QC - 1:
                    for t in range(4 * qc, 4 * (qc + 1)):
                        fillers.append(((qc, 1), lambda t=t: v_group(t)))
                # projection: qc0+t4,t5 during qc2; rest of qc1+qc2 during qc3
                for tp in {2: range(0, 6), 3: range(6, 12)}.get(qc, ()):
                    for f in proj_fillers(tp):
                        fillers.append((NODL, f))

                def mk_stage(p, qc=qc):
                    jq, jk = p, 2 + p

                    def stage(kb, pts):
                        """score matmuls + exp for both heads; on diagonal
                        blocks the dead triangle of exp(S^T) is zeroed with a
                        DVE tril-multiply (no PE mask matmuls)."""
                        m = kb - 4 * qc  # >= 0 on the diagonal chunk
                        lv = 128 * max(m, 0)  # first causally-live column
                        s_ps = sps.tile([128, 2, 512], F32, tag="spair")
                        for hi in range(2):
                            nc.tensor.matmul(
                                s_ps[:, hi, lv:512],
                                qkT_s[64 * hi:64 * (hi + 1), jk, 128 * kb:128 * (kb + 1)],
                                qkT_s[64 * hi:64 * (hi + 1), jq, 512 * qc + lv:512 * (qc + 1)],
                                start=True, stop=True,
                                tile_position=(64 * hi, 0), skip_group_check=True)
                        pt = ptpool.tile([128, 2, 512], BF16, tag="pt")
                        nc.scalar.activation(pt[:, :, lv:512], s_ps[:, :, lv:512],
                                             Exp, scale=SCALE)
                        if m >= 0:
                            for hi in range(2):
                                dd = pt[:, hi, lv:lv + 128]
                                nc.vector.tensor_mul(dd, dd, tril_s[:])
                        pts[kb] = (pt, lv)

                    return stage

                stage_fns = (mk_stage(0), mk_stage(1))
                ptsd = ({}, {})

                for p in range(2):
                    flush_due((qc, p))
                    nkb = 4 * qc + 4
                    ya = accps.tile([65, 512], F32, tag="acc")
                    yb = accps.tile([65, 512], F32, tag="acc")
                    pts = ptsd[p]
                    stage = stage_fns[p]

                    def consume(kb, p=p, ya=ya, yb=yb, nkb=nkb, pts=None):
                        pt, lv = pts.pop(kb)
                        for hi, y_ps in ((0, ya), (1, yb)):
                            nc.tensor.matmul(
                                y_ps[:, lv:512],
                                v_s[:, kb, 2 * p + hi, :],
                                pt[:, hi, lv:512],
                                start=(kb == 0), stop=(kb == nkb - 1),
                                skip_group_check=True)

                    DEPTH = 2
                    # (0,0): pre-stage ALL four score stages before v_group(0)
                    # so they aren't queued behind the late wv DMA; v_group(0)
                    # itself must precede consume(0) on the PE queue.
                    # p=1's first DEPTH stages were pre-staged at the tail of
                    # p=0 (the exp latency is absorbed by p0's norm window).
                    if p == 0:
                        pre = nkb if qc == 0 else min(DEPTH, nkb)
                        for kb in range(pre):
                            stage(kb, pts)
                        if qc == 0:
                            v_group(0)
                    else:
                        pre = min(DEPTH, nkb)
                    for kb in range(nkb):
                        if kb + DEPTH < nkb and kb + DEPTH >= pre:
                            stage(kb + DEPTH, pts)
                        consume(kb, pts=pts)
                        emit_filler(2 if qc == 0 else 1)
                        if p == 0 and kb == nkb - 1:
                            # pre-stage p1's first stages before p0's norm
                            for kb1 in range(min(DEPTH, nkb)):
                                stage_fns[1](kb1, ptsd[1])

                    # normalize + write y^T (head A -> partitions 0:64, B -> 64:128).
                    # Copies run inline (they free the accumulator PSUM banks);
                    # reciprocal -> broadcast DMA -> multiply are deferred into
                    # the next attention steps.
                    yc = npool.tile([65, 2, 512], BF16, tag="yc")
                    # casts gate the NEXT pair's first PV (accps ring reuse)
                    # AND the norm chain: high_priority schedules them ahead
                    # of same-time drain pops on the DVE queue. On the last
                    # chain one cast goes to ACT so they overlap.
                    with tc.high_priority():
                        if qc == QC - 1 and p == 1:
                            nc.scalar.copy(yc[:, 0, :], ya[:])
                        else:
                            nc.vector.tensor_copy(yc[:, 0, :], ya[:])
                        nc.vector.tensor_copy(yc[:, 1, :], yb[:])
                    rr = npool.tile([64, 2, 512], F32, tag="rr")
                    dps = {}

                    def norm_bcast(yc=yc, dps=dps):
                        # broadcast the partition-64 denominator row across 64
                        # partitions with a K=1 ones-matmul. (Alternatives all
                        # lose on HW: DVE can't read nonzero partition
                        # offsets, gpsimd partition_broadcast and DVE
                        # stream_shuffle are both catastrophically slow.)
                        for hi in range(2):
                            dp = mmps.tile([128, 512], F32, tag="mm", name="dps")
                            nc.tensor.matmul(
                                dp[0:64, :], ones_s[64:65, :], yc[64:65, hi, :],
                                start=True, stop=True,
                                tile_position=(64, 0), skip_group_check=True)
                            dps[hi] = dp

                    def norm_recip(rr=rr, dps=dps):
                        for hi in range(2):
                            nc.vector.reciprocal_approx_fast(
                                rr[:, hi, :], dps.pop(hi)[0:64, :])

                    def norm_mul(p=p, qc=qc, yc=yc, rr=rr):
                        # hi=0 on gpsimd (otherwise idle), hi=1 on DVE
                        nc.gpsimd.tensor_mul(
                            yT_s[0:64, p, 512 * qc:512 * (qc + 1)],
                            yc[0:64, 0, :], rr[:, 0, :])
                        nc.vector.tensor_mul(
                            yT_s[64:128, p, 512 * qc:512 * (qc + 1)],
                            yc[0:64, 1, :], rr[:, 1, :])

                    # insert the norm chain AFTER the next queued filler: the
                    # bcast matmul waits on the DVE yc-copies just queued, so
                    # one slot of independent PE work absorbs that latency
                    # instead of the PE stalling on it.
                    pos = 1 if fillers else 0
                    for fn in (norm_mul, norm_recip, norm_bcast):
                        fillers.insert(pos, (NODL, fn))

            # ---- epilogue: the final projection t-blocks run on the freed
            # attention PSUM pools. Their p2=0 halves depend only on yT pair
            # 0 (normalized during qc3-p1), so they execute DURING the last
            # norm chain — real PE work bridges the gap and keeps HAM warm.
            # p2=1 halves land after the last norm_mul writes yT pair 1.
            # t15 uses mmps, which the norm chain's bcast also cycles, so its
            # tiles must be allocated after the chain (emit_filler) to avoid
            # a ring-order deadlock.
            def eproj_mm(half_ap, t, p2, n2):
                nc.tensor.matmul(
                    half_ap,
                    yT_s[:, p2, 128 * t:128 * (t + 1)],
                    wp_s[:, p2, 512 * n2:512 * (n2 + 1)],
                    start=(p2 == 0), stop=(p2 == 1),
                )

            # pre-accumulation runs at background priority: the scheduler may
            # only use it to fill PE idle (the norm-chain window), never to
            # displace the attention critical path.
            eps = {}  # t -> (whole_or_None, [half0_ap, half1_ap])
            _pr = tc.cur_priority
            tc.cur_priority += 100000
            for t in (12, 13, 14):
                if t < 14:
                    ps2 = sps.tile([128, 2, 512], F32, tag="spair", name="eproj")
                    eps[t] = (ps2, [ps2[:, 0, :], ps2[:, 1, :]])
                else:
                    pa = accps.tile([128, 512], F32, tag="acc", name="eproj")
                    pb = accps.tile([128, 512], F32, tag="acc", name="eproj")
                    eps[t] = (None, [pa[:], pb[:]])
                for n2 in range(2):
                    eproj_mm(eps[t][1][n2], t, 0, n2)
            tc.cur_priority = _pr
            emit_filler(len(fillers))  # last norm chain (bcast/recip/mul)
            pa = mmps.tile([128, 512], F32, tag="mm", name="eproj")
            pb = mmps.tile([128, 512], F32, tag="mm", name="eproj")
            eps[15] = (None, [pa[:], pb[:]])
            for n2 in range(2):
                eproj_mm(eps[15][1][n2], 15, 0, n2)
            if dbg:
                nc.sync.dma_start(out=dbg_qkT.ap()[:], in_=qkT_s[:])
                nc.sync.dma_start(out=dbg_v.ap()[:], in_=v_s[:])
                nc.sync.dma_start(out=dbg_yT.ap()[:], in_=yT_s[:])
            outs = []
            for t in range(4 * (QC - 1), TQ):
                whole, halves = eps[t]
                for n2 in range(2):
                    eproj_mm(halves[n2], t, 1, n2)
                o_t = opool.tile([128, C], BF16, tag="o", name="o_t")
                if whole is not None:
                    if t % 2:
                        nc.scalar.copy(o_t[:], whole[:])
                    else:
                        nc.vector.tensor_copy(o_t[:], whole[:])
                else:
                    # half-drains split across ACT/DVE
                    for n2 in range(2):
                        dst = o_t[:, 512 * n2:512 * (n2 + 1)]
                        if (t + n2) % 2:
                            nc.scalar.copy(dst, halves[n2])
                        else:
                            nc.vector.tensor_copy(dst, halves[n2])
                outs.append((t, o_t))
            # all store issues AFTER all drains are emitted (a ~0.6us issue
            # between two drains on the same engine FIFO delays the second);
            # alternate rings so the four transfers overlap
            for t, o_t in outs:
                eng = nc.scalar if t % 2 else nc.sync
                eng.dma_start(out=out_d.ap()[128 * t:128 * (t + 1), :],
                              in_=o_t[:])

    nc.compile()
    return nc


def _shard_inputs(x, W_attn, b_attn, W_proj, b_proj):
    """Build the 8 per-core input maps (numpy, bf16 where applicable)."""
    pp = np.arange(128)[:, None]
    jj = np.arange(128)[None, :]
    # P^T[k, q] is causally dead for k > q within a diagonal block
    tril = np.where(pp <= jj, 1.0, 0.0).astype(NP_BF16)  # [128, 128]
    in_maps = []
    for c in range(N_CORES):
        b, g = c // 4, c % 4
        ch = slice(256 * g, 256 * (g + 1))
        wq = W_attn[:, ch]
        wk = W_attn[:, C:][:, ch]
        wv = W_attn[:, 2 * C:][:, ch]
        wqk = np.concatenate([wq, wk], axis=1).astype(NP_BF16)
        # [C, 512] -> [128, 4j, CT, 128]
        wqk = np.ascontiguousarray(
            wqk.reshape(CT, 128, 4, 128).transpose(1, 2, 0, 3))
        bq = b_attn[ch]
        bk = b_attn[C:][ch]
        bv = b_attn[2 * C:][ch]
        bqk = np.concatenate([bq, bk]).reshape(4, 128).T.astype(np.float32)  # [128, 4]
        xTc = (x[b].T.reshape(C, QC, 512).transpose(1, 0, 2)
               .reshape(QC, CT, 128, 512).transpose(0, 2, 1, 3))
        wvc = wv.astype(NP_BF16).reshape(CT, 128, 256).transpose(1, 0, 2)
        wpc = (W_proj[ch, :].astype(NP_BF16)
               .reshape(2, 128, C).transpose(1, 0, 2))
        in_maps.append({
            "xT": np.ascontiguousarray(xTc).astype(NP_BF16),
            "wqk": wqk,
            "wv": np.ascontiguousarray(wvc),
            "wp": np.ascontiguousarray(wpc),
            "bqk": np.ascontiguousarray(bqk),
            "bv": np.broadcast_to(bv.astype(NP_BF16), (128, 256)).copy(),
            "tril": tril,
        })
    return in_maps


def _run(in_maps, trace=False, **kw):
    global _compiled
    if _compiled is None:
        _compiled = _build_nc()
    return run_bass_kernel_spmd(_compiled, in_maps, list(range(N_CORES)),
                                trace=trace, **kw)


def kernel(x, W_attn, b_attn, W_proj, b_proj):
    x = np.asarray(x, dtype=np.float32)
    W_attn = np.asarray(W_attn, dtype=np.float32)
    b_attn = np.asarray(b_attn, dtype=np.float32)
    W_proj = np.asarray(W_proj, dtype=np.float32)
    b_proj = np.asarray(b_proj, dtype=np.float32)

    in_maps = _shard_inputs(x, W_attn, b_attn, W_proj, b_proj)
    res = _run(in_maps)
    out = np.zeros((B, T, C), dtype=np.float32)
    for c in range(N_CORES):
        out[c // 4] += np.asarray(res.results[c]["out"], dtype=np.float32)
    out += b_proj
    return out

